# revision 19
# baseline (speedup 1.0000x reference)
"""MHSA Bass kernel for TRN2, data-parallel over batch across 8 NeuronCores.

Problem: B=8, S=1024, D=768, H=12, DH=64.
  xh = x.reshape(B,S,H,DH); q/k/v = per-head Linear(xh); scores=q@k^T/8;
  out = softmax(scores) @ v, heads re-concatenated.

v3 design. ScalarE exp is the roofline (12 heads * 1024^2 scores / 128
lanes / 1.2 GHz ~= 82 us/core + per-instruction overhead ~= 100 us);
everything else must hide under it, and the PE must stay continuously
busy (any idle gap drops it to the 1.2 GHz p-state).

  - Softmax shift-invariance folds the k-projection away entirely:
      scores ~=~ (x @ (Wq Wk^T / 8) + bq Wk^T / 8) @ x^T
    so xT (bf16) doubles as kT and Wk/bk never reach the device.
  - x arrives as 16 half-width DMAs so pair-0 work starts ~5us in; DVE
    converts to bf16; PE transposes (1 cyc/row bf16) feed xT per pair.
  - Pair-outer pipeline exactly like v1: transposes+projections for
    pair c+1 are interleaved with attention for pair c, keeping PE
    dense. Per (pair,qb,t): two concurrent row-tiled QK^T matmuls
    (heads on partitions 0:64/64:128), one [128,1024] exp on ScalarE
    (PSUM->SBUF bf16; no max subtraction: |scores| < ~2 by
    construction), two PV matmuls accumulating out^T and sumexp via a
    ones column in V'; v-bias is folded into V rows (adds bv*sumexp,
    which normalization turns into +bv exactly).
  - Epilogue per (pair,qb,head): one PSUM->SBUF copy, 4 PE transposes
    into one packed PSUM tile, one batched reciprocal, 4 scaled copies
    into a [128, 4*64] staging tile, then a single DMA covering all
    512 q rows of that head (24 stores/iter instead of 96).
"""

import numpy as np

import concourse.bass as bass
import concourse.mybir as mybir
import concourse.tile as tile
from concourse import bacc
from concourse.bass_utils import run_bass_kernel_spmd
from concourse.masks import make_identity

B, S, D, H, DH = 8, 1024, 768, 12, 64
NP = H // 2  # head pairs
F32 = mybir.dt.float32
BF16 = mybir.dt.bfloat16
I16 = mybir.dt.int16
AF = mybir.ActivationFunctionType
ALU = mybir.AluOpType

# Schraudolph-style fast exp, emitting the bf16 bit pattern directly:
# bf16_bits(exp(x)) ~=~ int16(x * 128/ln2 + (127*128 - C)); C tuned for
# min max-rel-err (~3.3%) over |x|<3. One DVE tensor_scalar replaces one
# ScalarE activation for the t-tiles listed in FAST_T (shifts exp work
# from the critical ScalarE onto DVE slack; softmax renormalization
# absorbs most of the approximation error).
FEXP_A = 184.6649652337873
FEXP_B = 16250.5
FAST_T = (4,)


def _build_nc(reps=1, hw_loop=0):
    nc = bacc.Bacc(
        "TRN2", target_bir_lowering=False, debug=False, enable_asserts=False
    )
    x_d = nc.dram_tensor("x", [S, D], F32, kind="ExternalInput")
    wq_d = nc.dram_tensor("wq", [128, NP * 128], BF16, kind="ExternalInput")
    wv_d = nc.dram_tensor("wv", [128, NP * 128], BF16, kind="ExternalInput")
    bq_d = nc.dram_tensor("bq", [128, NP], F32, kind="ExternalInput")
    bvb_d = nc.dram_tensor("bvb", [128, NP * 128], F32, kind="ExternalInput")
    out_d = nc.dram_tensor("out", [S, D], F32, kind="ExternalOutput")

    from contextlib import ExitStack

    with tile.TileContext(nc) as tc, ExitStack() as ctx_pools:
        ps_s = ctx_pools.enter_context(tc.tile_pool(name="ps_s", bufs=2, space="PSUM"))
        ps_o = ctx_pools.enter_context(tc.tile_pool(name="ps_o", bufs=1, space="PSUM"))
        ps_t = ctx_pools.enter_context(tc.tile_pool(name="ps_t", bufs=2, space="PSUM"))
        sb_p = ctx_pools.enter_context(tc.tile_pool(name="sb_p", bufs=4))
        sb_o = ctx_pools.enter_context(tc.tile_pool(name="sb_o", bufs=3))
        sb_r = ctx_pools.enter_context(tc.tile_pool(name="sb_r", bufs=4))
        sb_y = ctx_pools.enter_context(tc.tile_pool(name="sb_y", bufs=3))
        with tc.tile_pool(name="persist", bufs=1) as pp:
            ident = pp.tile([128, 128], F32, tag="ident")
            make_identity(nc, ident[:])
            identb = pp.tile([128, 128], BF16, tag="identb")
            nc.vector.tensor_copy(identb[:], ident[:])
            # warm the exp table set at t=0 so the first real exp doesn't
            # pay the ~2.7us ACT_TABLE_LOAD on the critical path
            warm = pp.tile([1, 8], F32, tag="warm")
            nc.scalar.activation(warm[:], ident[0:1, 0:8], AF.Exp)

            wq_s = pp.tile([128, NP * 128], BF16, tag="wq")
            wv_s = pp.tile([128, NP * 128], BF16, tag="wv")
            bq_s = pp.tile([128, NP], F32, tag="bq")
            bvb_s = pp.tile([128, NP * 128], F32, tag="bvb")
            nc.sync.dma_start(wq_s[:], wq_d[:, :])
            nc.sync.dma_start(wv_s[:], wv_d[:, :])
            nc.sync.dma_start(bq_s[:], bq_d[:, :])
            nc.sync.dma_start(bvb_s[:], bvb_d[:, :])

            xf_all = pp.tile([128, 8 * D], F32, tag="xf")
            xb_all = pp.tile([128, 8 * D], BF16, tag="xb")
            xf = [xf_all[:, t * D : (t + 1) * D] for t in range(8)]
            xb = [xb_all[:, t * D : (t + 1) * D] for t in range(8)]
            # [p, t, c] views of x in DRAM and of the staging tiles, for
            # wide multi-tile DMAs
            x_d3 = x_d[:, :].rearrange("(t p) c -> p t c", p=128)
            xf3 = xf_all[:].rearrange("p (t c) -> p t c", t=8)
            xT = pp.tile([128, NP * 1024], BF16, tag="xT")
            qT = pp.tile([128, NP * 1024], BF16, tag="qT")
            vv = pp.tile([128, NP * 1040], BF16, tag="vv")
            # ones columns of V' (col 64 of each 65-wide block) are never
            # overwritten by the projection writes below
            nc.vector.memset(vv[:], 1.0)

            def phase1():
                # x arrives in 6 wide DMAs: one per (third of D, half of S),
                # each filling a 256-col slice of 4 s-tiles at once, so
                # pair 0/1 data lands ~3us in. DVE converts chase each DMA.
                for third in range(3):
                    cs = slice(third * 256, (third + 1) * 256)
                    for tg in range(2):
                        ts = slice(tg * 4, (tg + 1) * 4)
                        nc.sync.dma_start(xf3[:, ts, cs], x_d3[:, ts, cs])
                    for t in range(8):
                        # bf16 conversion on the otherwise-idle Pool engine
                        nc.gpsimd.tensor_copy(xb[t][:, cs], xf[t][:, cs])

            def transpose_pair(c):
                ps = ps_t.tile([128, 1024], BF16, tag="t")
                for t in range(8):
                    nc.tensor.transpose(
                        ps[:, t * 128 : (t + 1) * 128],
                        xb[t][:, c * 128 : (c + 1) * 128],
                        identb[:],
                    )
                nc.vector.tensor_copy(
                    xT[:, c * 1024 : (c + 1) * 1024], ps[:]
                )

            def phase2(c):
                cq = c * 1024
                wqc = wq_s[:, c * 128 : (c + 1) * 128]
                wvc = wv_s[:, c * 128 : (c + 1) * 128]
                for h2 in range(2):
                    qps = ps_t.tile([128, 512], F32, tag="t")
                    nc.tensor.matmul(
                        qps[:], wqc, xT[:, cq + h2 * 512 : cq + (h2 + 1) * 512],
                        start=True, stop=True,
                    )
                    nc.vector.tensor_scalar_add(
                        qT[:, cq + h2 * 512 : cq + (h2 + 1) * 512],
                        qps[:], bq_s[:, c : c + 1],
                    )
                bvc = bvb_s[:, c * 128 : (c + 1) * 128].rearrange(
                    "p (a b) -> p a b", a=2
                )
                for t in range(8):
                    vps = ps_t.tile([128, 128], F32, tag="t")
                    nc.tensor.matmul(
                        vps[:],
                        xT[:, cq + t * 128 : cq + (t + 1) * 128],
                        wvc,
                        start=True, stop=True,
                    )
                    base = c * 1040 + t * 130
                    dst = vv[:, base : base + 130].rearrange(
                        "p (a b) -> p a b", a=2
                    )[:, :, 0:64]
                    src = vps[:].rearrange("p (a b) -> p a b", a=2)
                    nc.vector.scalar_tensor_tensor(
                        dst, src, 0.0, bvc, ALU.add, ALU.add
                    )

            def epilogue(c, qb, oA, oB):
                # runs one (pair, qb) unit's normalization + store; emitted
                # lazily from inside the NEXT unit's t-loop so the PE-queue
                # epilogue transposes never delay the next unit's scores
                q0 = qb * 512
                for h_i, oT in ((0, oA), (1, oB)):
                    osb = sb_o.tile([65, 512], F32, tag="o")
                    nc.vector.tensor_copy(osb[:], oT[:])
                    tps = ps_t.tile([128, 260], F32, tag="t")
                    for j in range(4):
                        nc.tensor.transpose(
                            tps[:, j * 65 : (j + 1) * 65],
                            osb[:, j * 128 : (j + 1) * 128],
                            ident[0:65, 0:65],
                        )
                    # one DVE evacuation of the packed transposes, then the
                    # normalize (divide by sumexp in col 64 of each 65-block)
                    # runs on the idle Pool engine, SBUF to SBUF
                    tss = sb_r.tile([128, 260], F32, tag="r")
                    nc.vector.tensor_copy(tss[:], tps[:])
                    y = sb_y.tile([128, 256], F32, tag="y")
                    for j in range(4):
                        nc.gpsimd.normalize_recip(
                            y[:, j * 64 : (j + 1) * 64],
                            tss[:, j * 65 : j * 65 + 64],
                            tss[:, j * 65 + 64 : j * 65 + 65],
                        )
                    col = (2 * c + h_i) * 64
                    dst = out_d[q0 : q0 + 512, col : col + 64].rearrange(
                        "(j p) e -> p j e", j=4
                    )
                    nc.sync.dma_start(dst, y[:].rearrange("p (j e) -> p j e", j=4))

            def phase3(c, qb, pending):
                cq = c * 1024
                cv = c * 1040
                q0 = qb * 512
                oA = ps_o.tile([65, 512], F32, tag="oA")
                oB = ps_o.tile([65, 512], F32, tag="oB")
                for t in range(8):
                    sps = ps_s.tile([128, 1024], F32, tag="s")
                    nc.tensor.matmul(
                        sps[:, 0:512],
                        xT[0:64, cq + t * 128 : cq + (t + 1) * 128],
                        qT[0:64, cq + q0 : cq + q0 + 512],
                        start=True, stop=True,
                    )
                    nc.tensor.matmul(
                        sps[:, 512:1024],
                        xT[64:128, cq + t * 128 : cq + (t + 1) * 128],
                        qT[64:128, cq + q0 : cq + q0 + 512],
                        start=True, stop=True,
                    )
                    if t in FAST_T:
                        p16 = sb_p.tile([128, 1024], I16, tag="p")
                        nc.vector.tensor_scalar(
                            p16[:], sps[:], FEXP_A, FEXP_B, ALU.mult, ALU.add
                        )
                        pA = p16[:, 0:512].bitcast(BF16)
                        pB = p16[:, 512:1024].bitcast(BF16)
                    else:
                        p_sb = sb_p.tile([128, 1024], BF16, tag="p")
                        nc.scalar.activation(p_sb[:], sps[:], AF.Exp)
                        pA = p_sb[:, 0:512]
                        pB = p_sb[:, 512:1024]
                    nc.tensor.matmul(
                        oA[:],
                        vv[:, cv + t * 130 : cv + t * 130 + 65],
                        pA,
                        start=(t == 0), stop=(t == 7),
                        skip_group_check=True,
                    )
                    nc.tensor.matmul(
                        oB[:],
                        vv[:, cv + t * 130 + 65 : cv + t * 130 + 130],
                        pB,
                        start=(t == 0), stop=(t == 7),
                        skip_group_check=True,
                    )
                    if t == 1 and pending is not None:
                        epilogue(*pending)
                return (c, qb, oA, oB)

            def body():
                phase1()
                transpose_pair(0)
                phase2(0)
                pending = None
                for c in range(NP):
                    if c + 1 < NP:
                        transpose_pair(c + 1)
                        phase2(c + 1)
                    for qb in range(2):
                        pending = phase3(c, qb, pending)
                epilogue(*pending)

            def loop_cm():
                return tc.For_i(
                    0, hw_loop, 1,
                    hint_engines=(
                        mybir.EngineType.PE,
                        mybir.EngineType.Activation,
                        mybir.EngineType.DVE,
                        mybir.EngineType.SP,
                    ),
                )

            if hw_loop:
                with loop_cm():
                    body()
            else:
                for _ in range(reps):
                    body()
    nc.compile()
    return nc


_NC = None


def _get_nc():
    global _NC
    if _NC is None:
        _NC = _build_nc()
    return _NC


def _pack(Wq, bq, Wk, bk, Wv, bv):
    Wq = np.asarray(Wq, np.float32)
    Wk = np.asarray(Wk, np.float32)
    Wv = np.asarray(Wv, np.float32)
    bq = np.asarray(bq, np.float32)
    bv = np.asarray(bv, np.float32)
    scale = 1.0 / np.sqrt(np.float32(DH))
    wqb = np.zeros((128, NP * 128), np.float32)
    wvb = np.zeros((128, NP * 128), np.float32)
    bqc = np.zeros((128, NP), np.float32)
    bvb = np.zeros((128, NP * 128), np.float32)
    for c in range(NP):
        a, b = 2 * c, 2 * c + 1
        # fold Wk into the q side: scores = (x Wt + bt) x^T with
        # Wt = Wq Wk^T * scale, bt = Wk bq * scale
        wqb[0:64, c * 128 : c * 128 + 64] = (Wq[a] @ Wk[a].T) * scale
        wqb[64:128, c * 128 + 64 : c * 128 + 128] = (Wq[b] @ Wk[b].T) * scale
        wvb[0:64, c * 128 : c * 128 + 64] = Wv[a]
        wvb[64:128, c * 128 + 64 : c * 128 + 128] = Wv[b]
        bqc[:, c] = np.concatenate([Wk[a] @ bq[a], Wk[b] @ bq[b]]) * scale
        bvb[:, c * 128 : (c + 1) * 128] = np.concatenate([bv[a], bv[b]])[None, :]
    import ml_dtypes

    wqb = np.ascontiguousarray(wqb.astype(ml_dtypes.bfloat16))
    wvb = np.ascontiguousarray(wvb.astype(ml_dtypes.bfloat16))
    return wqb, wvb, bqc, bvb


def _in_maps(sequences, packed):
    wqb, wvb, bqc, bvb = packed
    return [
        {
            "x": np.ascontiguousarray(sequences[i]),
            "wq": wqb,
            "wv": wvb,
            "bq": bqc,
            "bvb": bvb,
        }
        for i in range(B)
    ]


def _run(sequences, Wq, bq, Wk, bk, Wv, bv, trace=False, tmpdir=None):
    sequences = np.ascontiguousarray(np.asarray(sequences, np.float32))
    packed = _pack(Wq, bq, Wk, bk, Wv, bv)
    nc = _get_nc()
    in_maps = _in_maps(sequences, packed)
    res = run_bass_kernel_spmd(
        nc, in_maps, core_ids=list(range(B)), trace=trace, tmpdir=tmpdir
    )
    out = np.stack([res.results[i]["out"] for i in range(B)], axis=0)
    return out, res


def kernel(sequences, Wq, bq, Wk, bk, Wv, bv):
    out, _ = _run(sequences, Wq, bq, Wk, bk, Wv, bv)
    return out


# revision 34
# speedup vs baseline: 1.1381x; 1.1381x over previous
"""MHSA Bass kernel for TRN2, data-parallel over batch across 8 NeuronCores.

Problem: B=8, S=1024, D=768, H=12, DH=64.
  xh = x.reshape(B,S,H,DH); q/k/v = per-head Linear(xh); scores=q@k^T/8;
  out = softmax(scores) @ v, heads re-concatenated.

Design notes. ScalarE exp is the roofline (12 heads * 1024^2 scores /
128 lanes / 1.2 GHz ~= 82 us/core + per-instruction overhead); the
engines are balanced around it:

  - Softmax shift-invariance folds the k-projection away entirely:
      scores ~=~ (x @ (Wq Wk^T / 8) + bq Wk^T / 8) @ x^T
    so xT (bf16) doubles as kT and Wk/bk never reach the device.
  - x arrives in 8 wide DMAs (few DMA instructions — the HWDGE charges
    ~625ns each — narrowest d-chunk first so pair-0 work starts ~4us
    in); bf16 converts run on the otherwise-idle Pool engine (chunk 0
    on DVE); PE transposes (1 cyc/row bf16) feed xT per pair.
  - Pair-outer software pipeline: transposes+projections for pair c+1
    interleave with attention for pair c, keeping PE dense (idle PE
    drops to the 1.2 GHz p-state). Per (pair,qb,t): two concurrent
    row-tiled QK^T matmuls (heads on partitions 0:64/64:128), one
    [128,1024] exp on ScalarE (PSUM->SBUF bf16; no max subtraction:
    |scores| < ~2 by construction), two PV matmuls accumulating out^T
    plus sumexp via a ones column in V'. v-bias is folded into V rows
    (adds bv*sumexp, which normalization turns into +bv exactly).
  - The last two k-tiles per unit use a Schraudolph-style fast exp on
    DVE instead of ScalarE (FAST_T below): one fused tensor_scalar
    emits the bf16 bit pattern as int16. Their later PV consumption
    hides the DVE latency; ~25% of exp work moves off the ScalarE
    critical path for ~0.25% relative error total.
  - Epilogue per (pair,qb) unit, emitted lazily from inside the next
    unit's t-loop: per head one DVE PSUM->SBUF copy, 4 PE transposes
    into one packed PSUM tile, one DVE evacuation, then the
    divide-by-sumexp runs as gpsimd.normalize_recip on Pool; one DMA
    per head covers all 512 q rows (24 stores/iter instead of 96).
"""

import numpy as np

import concourse.bass as bass
import concourse.mybir as mybir
import concourse.tile as tile
from concourse import bacc
from concourse.bass_utils import run_bass_kernel_spmd
from concourse.masks import make_identity

B, S, D, H, DH = 8, 1024, 768, 12, 64
NP = H // 2  # head pairs
F32 = mybir.dt.float32
BF16 = mybir.dt.bfloat16
I16 = mybir.dt.int16
AF = mybir.ActivationFunctionType
ALU = mybir.AluOpType

# Schraudolph-style fast exp, emitting the bf16 bit pattern directly:
# bf16_bits(exp(x)) ~=~ int16(x * 128/ln2 + (127*128 - C)); C tuned for
# min max-rel-err (~3.3%) over |x|<3. One DVE tensor_scalar replaces one
# ScalarE activation for the t-tiles listed in FAST_T (shifts exp work
# from the critical ScalarE onto DVE slack; softmax renormalization
# absorbs most of the approximation error).
FEXP_A = 184.6649652337873
FEXP_B = 16250.5
FAST_T = (6, 7)


def _build_nc(reps=1, hw_loop=0):
    nc = bacc.Bacc(
        "TRN2", target_bir_lowering=False, debug=False, enable_asserts=False
    )
    x_d = nc.dram_tensor("x", [S, D], F32, kind="ExternalInput")
    wqv_d = nc.dram_tensor("wqv", [128, 2 * NP * 128], BF16, kind="ExternalInput")
    bqv_d = nc.dram_tensor("bqv", [128, NP + NP * 128], F32, kind="ExternalInput")
    out_d = nc.dram_tensor("out", [S, D], F32, kind="ExternalOutput")

    from contextlib import ExitStack

    with tile.TileContext(nc) as tc, ExitStack() as ctx_pools:
        ps_s = ctx_pools.enter_context(tc.tile_pool(name="ps_s", bufs=2, space="PSUM"))
        ps_o = ctx_pools.enter_context(tc.tile_pool(name="ps_o", bufs=1, space="PSUM"))
        ps_t = ctx_pools.enter_context(tc.tile_pool(name="ps_t", bufs=2, space="PSUM"))
        sb_p = ctx_pools.enter_context(tc.tile_pool(name="sb_p", bufs=6))
        sb_o = ctx_pools.enter_context(tc.tile_pool(name="sb_o", bufs=3))
        sb_r = ctx_pools.enter_context(tc.tile_pool(name="sb_r", bufs=4))
        sb_y = ctx_pools.enter_context(tc.tile_pool(name="sb_y", bufs=3))
        with tc.tile_pool(name="persist", bufs=1) as pp:
            ident = pp.tile([128, 128], F32, tag="ident")
            make_identity(nc, ident[:])
            identb = pp.tile([128, 128], BF16, tag="identb")
            nc.gpsimd.tensor_copy(identb[:], ident[:])
            # warm the exp table set at t=0 so the first real exp doesn't
            # pay the ~2.7us ACT_TABLE_LOAD on the critical path
            warm = pp.tile([1, 8], F32, tag="warm")
            nc.scalar.activation(warm[:], ident[0:1, 0:8], AF.Exp)

            wqv_s = pp.tile([128, 2 * NP * 128], BF16, tag="wqv")
            bqv_s = pp.tile([128, NP + NP * 128], F32, tag="bqv")
            # weights on the ACT queue so x loads win the HWDGE race
            nc.scalar.dma_start(wqv_s[:], wqv_d[:, :])
            nc.scalar.dma_start(bqv_s[:], bqv_d[:, :])
            wq_s = wqv_s[:, 0 : NP * 128]
            wv_s = wqv_s[:, NP * 128 : 2 * NP * 128]
            bq_s = bqv_s[:, 0:NP]
            bvb_s = bqv_s[:, NP : NP + NP * 128]

            xf_all = pp.tile([128, 8 * D], F32, tag="xf")
            xb_all = pp.tile([128, 8 * D], BF16, tag="xb")
            xf = [xf_all[:, t * D : (t + 1) * D] for t in range(8)]
            xb = [xb_all[:, t * D : (t + 1) * D] for t in range(8)]
            # [p, t, c] views of x in DRAM and of the staging tiles, for
            # wide multi-tile DMAs
            x_d3 = x_d[:, :].rearrange("(t p) c -> p t c", p=128)
            xf3 = xf_all[:].rearrange("p (t c) -> p t c", t=8)
            xT = pp.tile([128, NP * 1024], BF16, tag="xT")
            qT = pp.tile([128, NP * 1024], BF16, tag="qT")
            vv = pp.tile([128, NP * 1040], BF16, tag="vv")
            # only the ones columns of V' (col 64 of each 65-wide block)
            # need initializing; the projections write everything else
            nc.vector.memset(
                vv[:].rearrange("p (x k) -> p x k", k=65)[:, :, 64], 1.0
            )

            def pe_warm():
                # no-dep matmuls that keep PE busy through the x-load
                # window so the p-state ramp is warm when real work lands
                wps = ps_t.tile([128, 512], F32, tag="t")
                for i in range(12):
                    nc.tensor.matmul(
                        wps[:, 0:128], ident[:], ident[:],
                        start=True, stop=True,
                    )

            def phase1():
                # x arrives in 8 wide DMAs, narrowest chunk first so
                # pair-0 work starts ~3us in; chunk-0 bf16 converts on DVE
                # (idle at start), the rest on the Pool engine
                for wi, (c0, c1) in enumerate(
                    [(0, 128), (128, 384), (384, 640), (640, 768)]
                ):
                    cs = slice(c0, c1)
                    for tg in range(2):
                        ts = slice(tg * 4, (tg + 1) * 4)
                        nc.sync.dma_start(xf3[:, ts, cs], x_d3[:, ts, cs])
                    eng = nc.vector if wi == 0 else nc.gpsimd
                    for t in range(8):
                        eng.tensor_copy(xb[t][:, cs], xf[t][:, cs])

            def transpose_pair(c):
                ps = ps_t.tile([128, 1024], BF16, tag="t")
                for t in range(8):
                    nc.tensor.transpose(
                        ps[:, t * 128 : (t + 1) * 128],
                        xb[t][:, c * 128 : (c + 1) * 128],
                        identb[:],
                    )
                nc.vector.tensor_copy(
                    xT[:, c * 1024 : (c + 1) * 1024], ps[:]
                )

            def phase2(c):
                cq = c * 1024
                wqc = wq_s[:, c * 128 : (c + 1) * 128]
                wvc = wv_s[:, c * 128 : (c + 1) * 128]
                for h2 in range(2):
                    qps = ps_t.tile([128, 512], F32, tag="t")
                    nc.tensor.matmul(
                        qps[:], wqc, xT[:, cq + h2 * 512 : cq + (h2 + 1) * 512],
                        start=True, stop=True,
                    )
                    nc.vector.tensor_scalar_add(
                        qT[:, cq + h2 * 512 : cq + (h2 + 1) * 512],
                        qps[:], bq_s[:, c : c + 1],
                    )
                bvc = bvb_s[:, c * 128 : (c + 1) * 128].rearrange(
                    "p (a b) -> p a b", a=2
                )
                for t in range(8):
                    vps = ps_t.tile([128, 128], F32, tag="t")
                    nc.tensor.matmul(
                        vps[:],
                        xT[:, cq + t * 128 : cq + (t + 1) * 128],
                        wvc,
                        start=True, stop=True,
                    )
                    base = c * 1040 + t * 130
                    dst = vv[:, base : base + 130].rearrange(
                        "p (a b) -> p a b", a=2
                    )[:, :, 0:64]
                    src = vps[:].rearrange("p (a b) -> p a b", a=2)
                    nc.vector.scalar_tensor_tensor(
                        dst, src, 0.0, bvc, ALU.add, ALU.add
                    )

            def epilogue(c, qb, oA, oB):
                # runs one (pair, qb) unit's normalization + store; emitted
                # lazily from inside the NEXT unit's t-loop so the PE-queue
                # epilogue transposes never delay the next unit's scores
                q0 = qb * 512
                for h_i, oT in ((0, oA), (1, oB)):
                    osb = sb_o.tile([65, 512], F32, tag="o")
                    nc.vector.tensor_copy(osb[:], oT[:])
                    tps = ps_t.tile([128, 260], F32, tag="t")
                    for j in range(4):
                        nc.tensor.transpose(
                            tps[:, j * 65 : (j + 1) * 65],
                            osb[:, j * 128 : (j + 1) * 128],
                            ident[0:65, 0:65],
                        )
                    # one DVE evacuation of the packed transposes, then the
                    # normalize (divide by sumexp in col 64 of each 65-block)
                    # runs on the idle Pool engine, SBUF to SBUF
                    tss = sb_r.tile([128, 260], F32, tag="r")
                    nc.vector.tensor_copy(tss[:], tps[:])
                    y = sb_y.tile([128, 256], F32, tag="y")
                    for j in range(4):
                        nc.gpsimd.normalize_recip(
                            y[:, j * 64 : (j + 1) * 64],
                            tss[:, j * 65 : j * 65 + 64],
                            tss[:, j * 65 + 64 : j * 65 + 65],
                        )
                    col = (2 * c + h_i) * 64
                    dst = out_d[q0 : q0 + 512, col : col + 64].rearrange(
                        "(j p) e -> p j e", j=4
                    )
                    nc.sync.dma_start(dst, y[:].rearrange("p (j e) -> p j e", j=4))

            def phase3(c, qb, pending):
                cq = c * 1024
                cv = c * 1040
                q0 = qb * 512
                oA = ps_o.tile([65, 512], F32, tag="oA")
                oB = ps_o.tile([65, 512], F32, tag="oB")
                for t in range(8):
                    sps = ps_s.tile([128, 1024], F32, tag="s")
                    nc.tensor.matmul(
                        sps[:, 0:512],
                        xT[0:64, cq + t * 128 : cq + (t + 1) * 128],
                        qT[0:64, cq + q0 : cq + q0 + 512],
                        start=True, stop=True,
                    )
                    nc.tensor.matmul(
                        sps[:, 512:1024],
                        xT[64:128, cq + t * 128 : cq + (t + 1) * 128],
                        qT[64:128, cq + q0 : cq + q0 + 512],
                        start=True, stop=True,
                    )
                    if t in FAST_T:
                        p16 = sb_p.tile([128, 1024], I16, tag="p")
                        nc.vector.tensor_scalar(
                            p16[:], sps[:], FEXP_A, FEXP_B, ALU.mult, ALU.add
                        )
                        pA = p16[:, 0:512].bitcast(BF16)
                        pB = p16[:, 512:1024].bitcast(BF16)
                    else:
                        p_sb = sb_p.tile([128, 1024], BF16, tag="p")
                        nc.scalar.activation(p_sb[:], sps[:], AF.Exp)
                        pA = p_sb[:, 0:512]
                        pB = p_sb[:, 512:1024]
                    nc.tensor.matmul(
                        oA[:],
                        vv[:, cv + t * 130 : cv + t * 130 + 65],
                        pA,
                        start=(t == 0), stop=(t == 7),
                        skip_group_check=True,
                    )
                    nc.tensor.matmul(
                        oB[:],
                        vv[:, cv + t * 130 + 65 : cv + t * 130 + 130],
                        pB,
                        start=(t == 0), stop=(t == 7),
                        skip_group_check=True,
                    )
                    if t == 1 and pending is not None:
                        epilogue(*pending)
                return (c, qb, oA, oB)

            def body():
                phase1()
                transpose_pair(0)
                phase2(0)
                pending = None
                for c in range(NP):
                    if c + 1 < NP:
                        transpose_pair(c + 1)
                        phase2(c + 1)
                    for qb in range(2):
                        pending = phase3(c, qb, pending)
                epilogue(*pending)

            def loop_cm():
                return tc.For_i(
                    0, hw_loop, 1,
                    hint_engines=(
                        mybir.EngineType.PE,
                        mybir.EngineType.Activation,
                        mybir.EngineType.DVE,
                        mybir.EngineType.SP,
                    ),
                )

            if hw_loop:
                with loop_cm():
                    for _ in range(reps):
                        body()
            else:
                for _ in range(reps):
                    body()
    nc.compile()
    return nc


_NC = None


def _get_nc():
    global _NC
    if _NC is None:
        _NC = _build_nc()
    return _NC


def _pack(Wq, bq, Wk, bk, Wv, bv):
    Wq = np.asarray(Wq, np.float32)
    Wk = np.asarray(Wk, np.float32)
    Wv = np.asarray(Wv, np.float32)
    bq = np.asarray(bq, np.float32)
    bv = np.asarray(bv, np.float32)
    scale = 1.0 / np.sqrt(np.float32(DH))
    wqb = np.zeros((128, NP * 128), np.float32)
    wvb = np.zeros((128, NP * 128), np.float32)
    bqc = np.zeros((128, NP), np.float32)
    bvb = np.zeros((128, NP * 128), np.float32)
    for c in range(NP):
        a, b = 2 * c, 2 * c + 1
        # fold Wk into the q side: scores = (x Wt + bt) x^T with
        # Wt = Wq Wk^T * scale, bt = Wk bq * scale
        wqb[0:64, c * 128 : c * 128 + 64] = (Wq[a] @ Wk[a].T) * scale
        wqb[64:128, c * 128 + 64 : c * 128 + 128] = (Wq[b] @ Wk[b].T) * scale
        wvb[0:64, c * 128 : c * 128 + 64] = Wv[a]
        wvb[64:128, c * 128 + 64 : c * 128 + 128] = Wv[b]
        bqc[:, c] = np.concatenate([Wk[a] @ bq[a], Wk[b] @ bq[b]]) * scale
        bvb[:, c * 128 : (c + 1) * 128] = np.concatenate([bv[a], bv[b]])[None, :]
    import ml_dtypes

    wqv = np.ascontiguousarray(
        np.concatenate([wqb, wvb], axis=1).astype(ml_dtypes.bfloat16)
    )
    bqv = np.ascontiguousarray(np.concatenate([bqc, bvb], axis=1))
    return wqv, bqv


def _in_maps(sequences, packed):
    wqv, bqv = packed
    return [
        {
            "x": np.ascontiguousarray(sequences[i]),
            "wqv": wqv,
            "bqv": bqv,
        }
        for i in range(B)
    ]


def _run(sequences, Wq, bq, Wk, bk, Wv, bv, trace=False, tmpdir=None):
    sequences = np.ascontiguousarray(np.asarray(sequences, np.float32))
    packed = _pack(Wq, bq, Wk, bk, Wv, bv)
    nc = _get_nc()
    in_maps = _in_maps(sequences, packed)
    res = run_bass_kernel_spmd(
        nc, in_maps, core_ids=list(range(B)), trace=trace, tmpdir=tmpdir
    )
    out = np.stack([res.results[i]["out"] for i in range(B)], axis=0)
    return out, res


def kernel(sequences, Wq, bq, Wk, bk, Wv, bv):
    out, _ = _run(sequences, Wq, bq, Wk, bk, Wv, bv)
    return out
